# revision 2
# baseline (speedup 1.0000x reference)
"""Trainium2 Bass kernel v2 for the 5-layer GraphConv GNN.

Design (per core, 8 cores, dst-partitioned: core owns NPC=12500 dst nodes):
- 8 GPSIMD "sets" per core, one per src-chunk of 12500 nodes. Set s's 16
  partitions hold the gather table chunk s (yT rows, f32, d=1) - the full
  table lives across the 128 partitions once, no replication.
- Edges grouped (set = src chunk, section = dst-node range of NS nodes),
  sorted by dst within a section, one leading dummy slot per section.
- Per section: ap_gather slots (Pool) -> in-place mult by edge weight (DVE)
  -> tensor_tensor_scan cumsum (DVE) -> ap_gather node endpoints (Pool)
  -> diff (DVE) into per-set partials P [128, ~NPC] bf16.
- Cross-set reduction + W application fused into PE matmuls:
  z = SEL^T @ P + W_root^T @ xT (PSUM accumulate), ACT relu+bias -> xT_next.
- Layer tables: L0 gathers raw x (6 dims); later layers gather
  y_l = x_l @ W_rel_l (dims 15,10,5,2), built on device + AllGather.
- Final layer: softmax over 2 = sigmoid of logit diff (saturates exactly).
"""

import sys
sys.path.insert(0, '/opt/trn_rl_repo')
import numpy as np
import ml_dtypes

N_NODES = 100000
N_CORES = 8
NPC = N_NODES // N_CORES          # 12500 dst nodes per core
NSETS = 8
CHUNK = N_NODES // NSETS          # 12500 src nodes per set-chunk
DIMS = [6, 20, 15, 10, 5, 2]
L = 5
GD = [6, 15, 10, 5, 2]            # gathered dims per layer
NSEC = 32
NS = (NPC + NSEC - 1) // NSEC     # nodes per section
NEPI = ((NS + 1 + 31) // 32) * 32  # endpoint idxs per section (32-aligned: Q7 reads idxs as 32-bit words, so per-section idx slices must stay 4B-aligned)
P_W = (NSEC - 1) * NS + NEPI - 1 + 1  # partials width (12560)

bf = ml_dtypes.bfloat16


def preprocess(edge_index, edge_weight):
    """Build per-core idx/w/endpoint arrays. Returns (idx16, eidx16, wtab, seclen)."""
    src = np.asarray(edge_index[0], dtype=np.int64)
    dst = np.asarray(edge_index[1], dtype=np.int64)
    w = np.asarray(edge_weight, dtype=np.float32)

    core = dst // NPC
    nloc = dst % NPC
    st = src // CHUNK
    sloc = (src - st * CHUNK).astype(np.int64)
    sec = nloc // NS
    nsec = nloc - sec * NS
    group = ((core * NSETS + st) * NSEC + sec).astype(np.int64)
    ngroups = N_CORES * NSETS * NSEC

    okey = group * NPC + nloc
    order = np.argsort(okey, kind='stable')
    g_s, sloc_s, w_s = group[order], sloc[order], w[order]

    counts = np.bincount(g_s, minlength=ngroups)
    seclen = int(counts.max()) + 1            # +1 dummy slot at pos 0
    seclen = ((seclen + 31) // 32) * 32       # 32-aligned (idx slice alignment)

    # slot positions within each group (dummy at 0)
    starts = np.concatenate([[0], np.cumsum(counts)[:-1]])
    pos = (np.arange(len(g_s)) - starts[g_s]) + 1

    idx_p = np.zeros((ngroups, seclen), np.int16)
    w_p = np.zeros((ngroups, seclen), np.float32)
    idx_p[g_s, pos] = sloc_s.astype(np.int16)
    w_p[g_s, pos] = w_s

    # endpoints: C[n] = 1-based position of last slot of node n (dummy-incl)
    gn_key = group * NS + nsec
    cnt_gn = np.bincount(gn_key, minlength=ngroups * NS).reshape(ngroups, NS)
    C = np.cumsum(cnt_gn, axis=1)             # [ngroups, NS]
    ep = np.zeros((ngroups, NEPI), np.int16)
    ep[:, 1:NS + 1] = C.astype(np.int16)
    if NS + 1 < NEPI:
        ep[:, NS + 1:] = C[:, -1:].astype(np.int16)

    # wrap into SBUF layouts per core
    def wrap16(a):
        # [NSETS, NSEC, M] -> [128, NSEC*M/16]: item j of (set s, sec) at
        # [16*s + j%16, sec*(M//16) + j//16]
        ns, nsec_, m = a.shape
        aw = a.reshape(ns, nsec_, m // 16, 16).transpose(0, 3, 1, 2)
        return np.ascontiguousarray(aw.reshape(ns * 16, nsec_ * (m // 16)))

    idx16 = np.zeros((N_CORES, 128, NSEC * seclen // 16), np.int16)
    eidx16 = np.zeros((N_CORES, 128, NSEC * NEPI // 16), np.int16)
    wtab = np.zeros((N_CORES, 128, NSEC * seclen), bf)
    for c in range(N_CORES):
        blk = slice(c * NSETS * NSEC, (c + 1) * NSETS * NSEC)
        idx_c = idx_p[blk].reshape(NSETS, NSEC, seclen)
        ep_c = ep[blk].reshape(NSETS, NSEC, NEPI)
        w_c = w_p[blk].reshape(NSETS, NSEC, seclen)
        idx16[c] = wrap16(idx_c)
        eidx16[c] = wrap16(ep_c)
        # w duplicated across the 16 partitions of each set, sections concat
        wtab[c] = np.repeat(
            w_c.reshape(NSETS, 1, NSEC * seclen), 16, axis=1
        ).reshape(128, NSEC * seclen).astype(bf)
    return idx16, eidx16, wtab, seclen


def build_gnn(nc, seclen, debug=False, reps=1):
    import concourse.tile as tile
    from concourse import mybir
    f32 = mybir.dt.float32
    bf16 = mybir.dt.bfloat16
    i16 = mybir.dt.int16
    AF = mybir.ActivationFunctionType
    OP = mybir.AluOpType

    SL = seclen
    # ---- DRAM I/O ----
    xtab_d = nc.dram_tensor("xtab", [128, CHUNK], f32, kind="ExternalInput")
    xT0_d = nc.dram_tensor("xT0", [DIMS[0], NPC], bf16, kind="ExternalInput")
    idx_d = nc.dram_tensor("idx16", [128, NSEC * SL // 16], i16, kind="ExternalInput")
    eidx_d = nc.dram_tensor("eidx16", [128, NSEC * NEPI // 16], i16, kind="ExternalInput")
    wtab_d = nc.dram_tensor("wtab", [128, NSEC * SL], bf16, kind="ExternalInput")
    sel_d = nc.dram_tensor("sel", [128, 16], bf16, kind="ExternalInput")
    dv_d = nc.dram_tensor("dv", [2, 1], bf16, kind="ExternalInput")
    selw0_d = nc.dram_tensor("selw0", [128, DIMS[1]], bf16, kind="ExternalInput")
    wrel_d = [nc.dram_tensor(f"wrel{l}", [DIMS[l], DIMS[l + 1]], bf16, kind="ExternalInput")
              for l in range(1, L)]
    wroot_d = [nc.dram_tensor(f"wroot{l}", [DIMS[l], DIMS[l + 1]], bf16, kind="ExternalInput")
               for l in range(L)]
    brel_d = [nc.dram_tensor(f"brel{l}", [DIMS[l + 1], 1], f32, kind="ExternalInput")
              for l in range(L)]
    out_d = nc.dram_tensor("out", [NPC, DIMS[L]], f32, kind="ExternalOutput")
    dbg = {}
    if debug:
        for l in range(L):
            dbg[f"P{l}"] = nc.dram_tensor(f"dbgP{l}", [128, P_W], bf16, kind="ExternalOutput")
            if l < L - 1:
                dbg[f"xT{l + 1}"] = nc.dram_tensor(
                    f"dbgxT{l + 1}", [DIMS[l + 1], NPC], bf16, kind="ExternalOutput")
        for sec in range(2):
            for nm in ("g", "m", "S"):
                dbg[f"{nm}{sec}"] = nc.dram_tensor(
                    f"dbg{nm}{sec}", [128, SL], f32, kind="ExternalOutput")
            dbg[f"E{sec}"] = nc.dram_tensor(
                f"dbgE{sec}", [128, NEPI], f32, kind="ExternalOutput")

    groups = [list(range(N_CORES))]
    NT = (NPC + 511) // 512        # z-assembly column tiles

    with tile.TileContext(nc) as tc:
        with (
            tc.tile_pool(name="const", bufs=1) as cpool,
            tc.tile_pool(name="table", bufs=1) as tpool,
            tc.tile_pool(name="xt", bufs=1) as xpool,
            tc.tile_pool(name="sect", bufs=2) as spool,
            tc.tile_pool(name="pp", bufs=1) as ppool_sb,
            tc.tile_pool(name="zps", bufs=2, space="PSUM") as zpool,
            tc.tile_pool(name="ysmall", bufs=3) as ypool,
            tc.tile_pool(name="dram", bufs=1, space="DRAM") as dpool,
        ):
            # ---- constants ----
            idx_sb = cpool.tile([128, NSEC * SL // 16], i16)
            nc.sync.dma_start(idx_sb[:], idx_d.ap()[:])
            eidx_sb = cpool.tile([128, NSEC * NEPI // 16], i16)
            nc.sync.dma_start(eidx_sb[:], eidx_d.ap()[:])
            sel_sb = cpool.tile([128, 16], bf16)
            nc.sync.dma_start(sel_sb[:], sel_d.ap()[:])
            dv_sb = cpool.tile([2, 1], bf16)
            nc.sync.dma_start(dv_sb[:], dv_d.ap()[:])
            selw0_sb = cpool.tile([128, DIMS[1]], bf16)
            nc.sync.dma_start(selw0_sb[:], selw0_d.ap()[:])
            wrel_sb = {}
            for i, l in enumerate(range(1, L)):
                t = cpool.tile([DIMS[l], DIMS[l + 1]], bf16, name=f"wrel_sb{l}")
                nc.sync.dma_start(t[:], wrel_d[i].ap()[:])
                wrel_sb[l] = t
            wroot_sb, brel_sb = [], []
            for l in range(L):
                t = cpool.tile([DIMS[l], DIMS[l + 1]], bf16, name=f"wroot_sb{l}")
                nc.sync.dma_start(t[:], wroot_d[l].ap()[:])
                wroot_sb.append(t)
                t2 = cpool.tile([DIMS[l + 1], 1], f32, name=f"brel_sb{l}")
                nc.sync.dma_start(t2[:], brel_d[l].ap()[:])
                brel_sb.append(t2)

            def emit_body(rep):
              xT_cur = xpool.tile([DIMS[0], NPC], bf16, name=f"xT0_{rep}", tag="xT", bufs=2)
              nc.sync.dma_start(xT_cur[:], xT0_d.ap()[:])

              for l in range(L):
                cg = GD[l]
                din = DIMS[l]
                cout = DIMS[l + 1]
                # ---- gather table ----
                table_sb = tpool.tile([128, CHUNK], f32, name=f"table{rep}_{l}", tag="tab")
                if l == 0:
                    nc.sync.dma_start(table_sb[:], xtab_d.ap()[:])
                else:
                    ybounce = dpool.tile([cg, NPC], f32, name=f"ybounce{rep}_{l}")
                    for t in range(NT):
                        c0, c1 = t * 512, min((t + 1) * 512, NPC)
                        yps = zpool.tile([cg, 512], f32, name=f"yps{rep}_{l}_{t}", tag="yps")
                        nc.tensor.matmul(
                            out=yps[:, :c1 - c0],
                            lhsT=wrel_sb[l][:],
                            rhs=xT_cur[:, c0:c1],
                            start=True, stop=True,
                        )
                        ysb = ypool.tile([cg, 512], f32, name=f"ysb{rep}_{l}_{t}", tag="ysb")
                        nc.scalar.activation(
                            out=ysb[:, :c1 - c0], in_=yps[:, :c1 - c0],
                            func=AF.Copy)
                        nc.sync.dma_start(ybounce[:, c0:c1], ysb[:, :c1 - c0])
                    ytab = dpool.tile([NSETS * cg, NPC], f32, name=f"ytab{rep}_{l}")
                    nc.gpsimd.collective_compute(
                        "AllGather", mybir.AluOpType.bypass,
                        replica_groups=groups,
                        ins=[ybounce[:].opt()],
                        outs=[ytab[:].opt()],
                    )
                    for s in range(NSETS):
                        nc.sync.dma_start(
                            table_sb[16 * s:16 * s + cg, :],
                            ytab[s * cg:(s + 1) * cg, :])

                # ---- edge grind ----
                P = ppool_sb.tile([128, P_W], bf16, name=f"P{rep}_{l}", tag="P")
                for sec in range(NSEC):
                    w_t = spool.tile([128, SL], bf16, name=f"w{rep}_{l}_{sec}", tag="w")
                    nc.sync.dma_start(w_t[:], wtab_d.ap()[:, sec * SL:(sec + 1) * SL])
                    gth = spool.tile([128, SL], f32, name=f"g{rep}_{l}_{sec}", tag="g")
                    nc.gpsimd.ap_gather(
                        out_ap=gth[:], in_ap=table_sb[:],
                        idxs_ap=idx_sb[:, sec * (SL // 16):(sec + 1) * (SL // 16)],
                        channels=128, num_elems=CHUNK, d=1, num_idxs=SL,
                    )
                    msg = spool.tile([128, SL], f32, name=f"m{rep}_{l}_{sec}", tag="m")
                    nc.vector.tensor_tensor(
                        out=msg[:], in0=gth[:], in1=w_t[:], op=OP.mult)
                    S = spool.tile([128, SL], f32, name=f"S{rep}_{l}_{sec}", tag="S")
                    nc.vector.tensor_tensor_scan(
                        out=S[:], data0=msg[:], data1=msg[:], initial=0.0,
                        op0=OP.add, op1=OP.bypass)
                    E = spool.tile([128, NEPI], f32, name=f"E{rep}_{l}_{sec}", tag="E")
                    nc.gpsimd.ap_gather(
                        out_ap=E[:], in_ap=S[:],
                        idxs_ap=eidx_sb[:, sec * (NEPI // 16):(sec + 1) * (NEPI // 16)],
                        channels=128, num_elems=SL, d=1, num_idxs=NEPI,
                    )
                    nc.vector.tensor_tensor(
                        out=P[:, sec * NS:sec * NS + NEPI - 1],
                        in0=E[:, 1:NEPI], in1=E[:, 0:NEPI - 1], op=OP.subtract)
                    if debug and l == 0 and sec < 2:
                        nc.sync.dma_start(dbg[f"g{sec}"].ap()[:], gth[:])
                        nc.sync.dma_start(dbg[f"m{sec}"].ap()[:], msg[:])
                        nc.sync.dma_start(dbg[f"S{sec}"].ap()[:], S[:])
                        nc.sync.dma_start(dbg[f"E{sec}"].ap()[:], E[:])
                if debug:
                    nc.sync.dma_start(dbg[f"P{l}"].ap()[:], P[:])

                # ---- z assembly (per 512-column tile) ----
                if l < L - 1:
                    xT_next = xpool.tile([cout, NPC], bf16, name=f"xT{rep}_{l + 1}",
                                         tag="xT", bufs=2)
                else:
                    xT_next = None
                for t in range(NT):
                    c0, c1 = t * 512, min((t + 1) * 512, NPC)
                    cw = c1 - c0
                    zps = zpool.tile([cout, 512], f32, name=f"z{rep}_{l}_{t}", tag="zps")
                    nc.tensor.matmul(
                        out=zps[:, :cw],
                        lhsT=(selw0_sb if l == 0 else sel_sb[:, :cg])[:],
                        rhs=P[:, c0:c1], start=True, stop=False)
                    nc.tensor.matmul(
                        out=zps[:, :cw], lhsT=wroot_sb[l][:],
                        rhs=xT_cur[:, c0:c1], start=False, stop=True)
                    if l < L - 1:
                        nc.scalar.activation(
                            out=xT_next[:, c0:c1], in_=zps[:, :cw],
                            func=AF.Relu, bias=brel_sb[l][:])
                    else:
                        # relu, then softmax over 2 = sigmoid(z0 - z1)
                        zrt = ypool.tile([2, 512], bf16, name=f"zrt{rep}_{t}", tag="zrt")
                        nc.scalar.activation(
                            out=zrt[:, :cw], in_=zps[:, :cw],
                            func=AF.Relu, bias=brel_sb[l][:])
                        dps = zpool.tile([1, 512], f32, name=f"dps{rep}_{t}", tag="dps")
                        nc.tensor.matmul(
                            out=dps[:, :cw], lhsT=dv_sb[:],
                            rhs=zrt[:, :cw], start=True, stop=True)
                        p0 = ypool.tile([1, 512], f32, name=f"p0_{rep}_{t}", tag="p0")
                        nc.scalar.activation(out=p0[:, :cw], in_=dps[:, :cw],
                                             func=AF.Sigmoid)
                        p1 = ypool.tile([1, 512], f32, name=f"p1_{rep}_{t}", tag="p1")
                        nc.scalar.activation(out=p1[:, :cw], in_=dps[:, :cw],
                                             func=AF.Sigmoid, scale=-1.0)
                        nc.sync.dma_start(
                            out_d.ap()[c0:c1, 0:1].rearrange("n o -> o n"), p0[:, :cw])
                        nc.sync.dma_start(
                            out_d.ap()[c0:c1, 1:2].rearrange("n o -> o n"), p1[:, :cw])
                if debug and l < L - 1:
                    nc.sync.dma_start(dbg[f"xT{l + 1}"].ap()[:], xT_next[:])
                xT_cur = xT_next

            for rep in range(reps):
                emit_body(rep)
    return dbg


def make_host_inputs(inputs):
    x = np.asarray(inputs["x"], np.float32)
    idx16, eidx16, wtab, seclen = preprocess(
        inputs["edge_index"], inputs["edge_weight"])
    xtab = np.zeros((128, CHUNK), np.float32)
    for s in range(NSETS):
        xtab[16 * s:16 * s + 6, :] = x[s * CHUNK:(s + 1) * CHUNK, :].T
    sel = np.zeros((128, 16), bf)
    for p in range(128):
        sel[p, p % 16] = 1
    dv = np.array([[1.0], [-1.0]], np.float32).astype(bf)
    wrel0 = np.asarray(inputs["w_rel0"], np.float32)   # [6, 20]
    wrel0_pad = np.zeros((16, DIMS[1]), np.float32)
    wrel0_pad[:6] = wrel0
    selw0 = (sel.astype(np.float32) @ wrel0_pad).astype(bf)

    common = {"xtab": xtab, "sel": sel, "selw0": selw0, "dv": dv}
    for l in range(1, L):
        common[f"wrel{l}"] = np.asarray(inputs[f"w_rel{l}"], np.float32).astype(bf)
    for l in range(L):
        common[f"wroot{l}"] = np.asarray(inputs[f"w_root{l}"], np.float32).astype(bf)
        common[f"brel{l}"] = np.asarray(inputs[f"b_rel{l}"], np.float32).reshape(-1, 1)
    in_maps = []
    for c in range(N_CORES):
        m = dict(common)
        m["idx16"] = idx16[c]
        m["eidx16"] = eidx16[c]
        m["wtab"] = wtab[c]
        m["xT0"] = np.ascontiguousarray(
            x[c * NPC:(c + 1) * NPC, :].T).astype(bf)
        in_maps.append(m)
    return in_maps, seclen


def _install_loud_hook():
    import traceback
    from concourse import bass2jax
    bass2jax.install_neuronx_cc_hook()
    try:
        import libneuronxla
    except ImportError:
        return
    hook = libneuronxla.neuronx_cc
    def loud(*a, **k):
        try:
            return hook(*a, **k)
        except BaseException:
            traceback.print_exc()
            raise
    libneuronxla.neuronx_cc = loud
    bass2jax.install_neuronx_cc_hook = lambda: None


def run_gnn(inputs, trace=False, debug=False, reps=1):
    import concourse.bacc as bacc
    from concourse.bass_utils import run_bass_kernel_spmd
    _install_loud_hook()
    in_maps, seclen = make_host_inputs(inputs)
    nc = bacc.Bacc("TRN2", target_bir_lowering=False, debug=False,
                   num_devices=N_CORES)
    build_gnn(nc, seclen, debug=debug, reps=reps)
    nc.compile()
    res = run_bass_kernel_spmd(nc, in_maps, core_ids=list(range(N_CORES)),
                               trace=trace)
    out = np.concatenate([res.results[k]["out"] for k in range(N_CORES)], axis=0)
    return out, res


def kernel(**inputs):
    out, _ = run_gnn(inputs)
    return out


def kernel_traced(**inputs):
    """Returns (out, BassKernelResults). exec_time_ns is None when NTFF
    profiling is unavailable (axon client without the hook)."""
    return run_gnn(inputs, trace=True)
